# revision 1
# baseline (speedup 1.0000x reference)
"""Joint soft-histogram kernel for Trainium2 (Bass/Tile), 8-core data parallel.

Math (per batch b, K=256, L=1/256, W=L/2.5, N=65536 pixels):
    phi_k(x) = sigmoid((x - k*L)/W) - sigmoid((x - (k+1)*L)/W)
             = S_k(x) - S_{k+1}(x),   S_k(x) = sigmoid(640*x - 2.5*k)
    out[k, j] = sum_n phi_k(x_n) * phi_j(y_n) / N

Half-telescope: out[k, j] = (H[k, j] - H[k, j+1]) / N with H = Phi_x^T @ T_y,
T_y[n, j] = S_j(y_n), j = 0..256. Only the x side needs the adjacent
difference before the matmul; the y-side difference collapses onto the tiny
(256, 257) H. H entries stay O(256), so fp32 PSUM accumulation is safe, and
fp16 Phi/T operands give ~3e-4 relative error overall.

Pipeline per 16-chunk group (chunk = 128 pixels on partitions):
  - DVE tensor_scalar: A = krow + 640*x_col  (pre-activation, fp32)
  - ScalarE one big-free-dim sigmoid over the group (amortizes the ~224cyc
    per-instruction overhead)
  - DVE (or GPSIMD) adjacent diff -> phi (fp16)
  - TensorE: 2 matmuls per chunk accumulate H in PSUM.
The y side can either be staged the same way (no diff needed) or computed
with one per-chunk activation using the per-partition bias operand.

Sharding: pure data parallel, batch b -> core b.
"""

import numpy as np

import concourse.bass as bass
import concourse.tile as tile
from concourse import bacc, mybir
from concourse.bass_utils import run_bass_kernel_spmd

F32 = mybir.dt.float32
F16 = mybir.dt.float16

B = 8
K = 256
KB = K + 1            # 257 bins on the S/T axis
KP = KB + 1           # 258: padded per-chunk stride; even innermost dim is
                      # required for the DVE tensor_scalar 2x_2P perf mode
NPIX = 65536
NCHUNK = 512
XG = 16               # chunks per staged group
NG = NCHUNK // XG     # 32 groups
GF = XG * KP          # staged group free size (4128)
SCALE = 640.0
INV_N = 1.0 / NPIX

# --- tuning knobs -----------------------------------------------------------
# stage the y side (DVE pre-add + big ACT) for group g, else per-chunk ACT-bias
Y_STAGED = [True] * NG
# engine for the x-side adjacent diff per group: 'v' (vector) or 'g' (gpsimd)
# gpsimd TT measured ~3x slower than DVE; give it ~19/32 groups so
# DVE(pre-adds + 13 diffs) ~ GPSIMD(19 diffs)
DIFF_ENG = ['g' if (g % 5) != 2 else 'v' for g in range(NG)]
DIFF_ENG = ['v' if (g % 3) == 0 else 'g' for g in range(NG)]
# ---------------------------------------------------------------------------

_cached_nc = None


def _build():
    # Bacc (not plain Bass): its finalize() runs generate_event_semaphores,
    # which splits multi-wait instructions that TRN2 engines cannot encode.
    nc = bacc.Bacc("TRN2")
    xd = nc.declare_dram_parameter("x", [128, 512], F32, isOutput=False)
    yd = nc.declare_dram_parameter("y", [128, 512], F32, isOutput=False)
    kd = nc.declare_dram_parameter("krow", [128, KP], F32, isOutput=False)
    od = nc.declare_dram_parameter("out", [256, 256], F32, isOutput=True)

    sig = mybir.ActivationFunctionType.Sigmoid
    add = mybir.AluOpType.add

    with tile.TileContext(nc) as tc:
        with (
            tc.tile_pool(name="singles", bufs=1) as singles,
            tc.tile_pool(name="stage32", bufs=2) as stage32,
            tc.tile_pool(name="stage16", bufs=3) as stage16,
            tc.tile_pool(name="work", bufs=4) as work,
            tc.tile_pool(name="psum", bufs=1, space="PSUM") as psum,
        ):
            xt = singles.tile([128, 512], F32)
            nc.sync.dma_start(out=xt, in_=xd[:, :])
            yt = singles.tile([128, 512], F32)
            nc.sync.dma_start(out=yt, in_=yd[:, :])
            kr = singles.tile([128, KP], F32)
            nc.sync.dma_start(out=kr, in_=kd[:, :])

            x6 = singles.tile([128, 512], F32)
            nc.vector.tensor_scalar_mul(out=x6, in0=xt, scalar1=SCALE)
            y6 = singles.tile([128, 512], F32)
            nc.vector.tensor_scalar_mul(out=y6, in0=yt, scalar1=SCALE)

            H = psum.tile([128, 2, 512], F32)

            for g in range(NG):
                c0 = g * XG
                # ---- x side: staged pre-add + one big sigmoid + group diff
                ax = stage32.tile([128, GF], F32, tag="ax")
                for i in range(XG):
                    nc.vector.tensor_scalar(
                        out=ax[:, i * KP : (i + 1) * KP],
                        in0=kr,
                        scalar1=x6[:, c0 + i : c0 + i + 1],
                        scalar2=None,
                        op0=add,
                    )
                sx = stage16.tile([128, GF], F16, tag="sx")
                nc.scalar.activation(out=sx, in_=ax, func=sig)
                ph = stage16.tile([128, GF], F16, tag="ph")
                if DIFF_ENG[g] == 'g':
                    nc.gpsimd.tensor_sub(
                        out=ph[:, 0 : GF - 1], in0=sx[:, 0 : GF - 1],
                        in1=sx[:, 1:GF],
                    )
                else:
                    nc.vector.tensor_sub(
                        out=ph[:, 0 : GF - 1], in0=sx[:, 0 : GF - 1],
                        in1=sx[:, 1:GF],
                    )

                # ---- y side
                if Y_STAGED[g]:
                    ay = stage32.tile([128, GF], F32, tag="ay")
                    for i in range(XG):
                        nc.vector.tensor_scalar(
                            out=ay[:, i * KP : (i + 1) * KP],
                            in0=kr,
                            scalar1=y6[:, c0 + i : c0 + i + 1],
                            scalar2=None,
                            op0=add,
                        )
                    tyg = stage16.tile([128, GF], F16, tag="tyg")
                    nc.scalar.activation(out=tyg, in_=ay, func=sig)

                # ---- matmuls
                for i in range(XG):
                    c = c0 + i
                    if Y_STAGED[g]:
                        ty = tyg[:, i * KP : i * KP + KB]
                    else:
                        tyt = work.tile([128, KB], F16, tag="ty")
                        nc.scalar.activation(
                            out=tyt, in_=kr[:, 0:KB], func=sig,
                            bias=y6[:, c : c + 1], scale=1.0,
                        )
                        ty = tyt
                    first = c == 0
                    last = c == NCHUNK - 1
                    nc.tensor.matmul(
                        H[:, 0, 0:KB],
                        lhsT=ph[:, i * KP : i * KP + 128],
                        rhs=ty,
                        start=first,
                        stop=last,
                    )
                    nc.tensor.matmul(
                        H[:, 1, 0:KB],
                        lhsT=ph[:, i * KP + 128 : i * KP + 256],
                        rhs=ty,
                        start=first,
                        stop=last,
                    )

            for h in range(2):
                t1 = work.tile([128, KB], F32, tag="ep")
                nc.scalar.activation(
                    out=t1,
                    in_=H[:, h, 0:KB],
                    func=mybir.ActivationFunctionType.Copy,
                    scale=INV_N,
                )
                t2 = work.tile([128, K], F32, tag="ep2")
                nc.vector.tensor_sub(out=t2, in0=t1[:, 0:K], in1=t1[:, 1:KB])
                nc.sync.dma_start(out=od[128 * h : 128 * (h + 1), :], in_=t2)

    nc.finalize()
    return nc


def _get_nc():
    global _cached_nc
    if _cached_nc is None:
        _cached_nc = _build()
    return _cached_nc


def _krow():
    row = np.arange(KP, dtype=np.float32) * np.float32(-2.5)
    return np.tile(row[None, :], (128, 1))


def _in_maps(x, y):
    x = np.ascontiguousarray(np.asarray(x, dtype=np.float32))
    y = np.ascontiguousarray(np.asarray(y, dtype=np.float32))
    kr = _krow()
    return [
        {
            "x": x[b].reshape(128, 512),
            "y": y[b].reshape(128, 512),
            "krow": kr,
        }
        for b in range(B)
    ]


def run(x, y, trace=False, **trace_kw):
    """Run on all 8 cores; returns (out (8,256,256) f32, BassKernelResults)."""
    nc = _get_nc()
    res = run_bass_kernel_spmd(nc, _in_maps(x, y), list(range(B)), trace=trace,
                               **trace_kw)
    out = np.stack([res.results[b]["out"] for b in range(B)]).astype(np.float32)
    return out, res


def kernel(x, y):
    out, _ = run(x, y)
    return out



# revision 2
# speedup vs baseline: 1.0025x; 1.0025x over previous
"""Joint soft-histogram kernel v3 for Trainium2 (Bass/Tile), 8-core data parallel.

Math (per batch b, K=256, N=65536 pixels):
    g_j(v) = tanh((640v - 2.5j)/2) = 2*sigmoid(640v - 2.5j) - 1
    M[k,j] = sum_n (g_k - g_{k+1})(x_n) * g_j(y_n)   (PSUM, fp16 operands)
    out[k,j] = (M[k,j] - M[k,j+1]) / (4N)

Layout: pixels on partitions, i-major group tiles [128, XG=16, 257] so matmul
operands are contiguous. Per group & side: one fp32 DVE tensor_tensor builds
arg = 640v - 2.5j (broadcast APs), one big ScalarE ACT tanh(0.5*arg) -> fp16,
x side gets an fp16 2x-rate adjacent diff. 2 matmuls per chunk accumulate M.

Sharding: pure data parallel, batch b -> core b.
"""

import numpy as np

import concourse.bass as bass
import concourse.tile as tile
from concourse import bacc, mybir
from concourse.bass_utils import run_bass_kernel_spmd

F32 = mybir.dt.float32
F16 = mybir.dt.float16

B = 8
K = 256
KB = K + 1
NPIX = 65536
NCHUNK = 512
XG = 16
NG = NCHUNK // XG
SCALE = 640.0
INV = 1.0 / (4.0 * NPIX)

QC0, QC1, QC2 = 0.46564883, -0.02206071, 0.00048341

# --- tuning knobs -----------------------------------------------------------
# per (side, group) sigma path: 'a' = ScalarE ACT tanh, 'q' = custom DVE quintic
SIG_PATH = {}
for g in range(NG):
    SIG_PATH[('x', g)] = 'a'
    SIG_PATH[('y', g)] = 'a'
# per-chunk ACT path (SE-only, no DVE expansion): dict (side, g) -> bool
PCHUNK = {}
for g in range(NG):
    PCHUNK[('y', g)] = (g % 2 == 0) and g != 30
    PCHUNK[('x', g)] = False
# engine for the x-side diff per group: 'v' (vector) or 'g' (gpsimd)
DIFF_ENG = ['v'] * NG
# ---------------------------------------------------------------------------

_cached_nc = None
_cache_key = None
_tanh_op = None


def _register_tanh_half_op():
    global _tanh_op
    if _tanh_op is not None:
        return _tanh_op
    import concourse.dve_ops as dvo
    from concourse.dve_spec import Spec, Src0, C0, C1, C2, Zero, One, sq, maxx, minn, lower
    from concourse.dve_uop import DveOpSpec

    NAME = "TANH_HALF_QUINT_ANT"
    for op in dvo.OPS:
        if op.name == NAME:
            _tanh_op = op
            return op

    t = sq(Src0)
    poly = Src0 * (C0 + t * (C1 + C2 * t))
    body = minn(maxx(poly, Zero - One), One)

    def _ref(in0, in1, s0, s1, imm2):
        tt = in0 * in0
        return np.clip(in0 * (s0 + tt * (s1 + imm2 * tt)), -1.0, 1.0)

    spec = Spec(body=body, reference=_ref)
    shas = {}
    for ver in ("v3", "v4"):
        uops = lower(spec, ver=ver)
        shas[ver] = DveOpSpec(name=NAME, opcode=1, uops=uops, rd1_en=False).sha(ver)
    op = dvo.DveOp(NAME, spec, subdim=False, uops_sha=shas)
    dvo.OPS.append(op)
    dvo._SUB_OPCODE_FOR_NAME[NAME] = dvo._CUSTOM_DVE_ROW_BASE + len(dvo.OPS) - 1
    assert dvo._SUB_OPCODE_FOR_NAME[NAME] < 0x20
    _tanh_op = op
    return op


def _build():
    op_q = _register_tanh_half_op()
    nc = bacc.Bacc("TRN2")
    xd = nc.declare_dram_parameter("x", [128, 512], F32, isOutput=False)
    yd = nc.declare_dram_parameter("y", [128, 512], F32, isOutput=False)
    kd = nc.declare_dram_parameter("krow", [128, KB], F32, isOutput=False)
    khd = nc.declare_dram_parameter("krw16h", [128, KB], F16, isOutput=False)
    od = nc.declare_dram_parameter("out", [256, 256], F32, isOutput=True)

    tanh = mybir.ActivationFunctionType.Tanh
    sub = mybir.AluOpType.subtract

    with tile.TileContext(nc) as tc:
        with (
            tc.tile_pool(name="singles", bufs=1) as singles,
            tc.tile_pool(name="args", bufs=4) as args,
            tc.tile_pool(name="gs", bufs=4) as gs,
            tc.tile_pool(name="phs", bufs=3) as phs,
            tc.tile_pool(name="work", bufs=2) as work,
            tc.tile_pool(name="psum", bufs=1, space="PSUM") as psum,
        ):
            xt = singles.tile([128, 512], F32)
            nc.sync.dma_start(out=xt, in_=xd[:, :])
            yt = singles.tile([128, 512], F32)
            nc.sync.dma_start(out=yt, in_=yd[:, :])
            krw = singles.tile([128, KB], F32)
            nc.sync.dma_start(out=krw, in_=kd[:, :])
            krw16h = singles.tile([128, KB], F16)
            nc.sync.dma_start(out=krw16h, in_=khd[:, :])

            x6 = singles.tile([128, 512], F32)
            nc.vector.tensor_scalar_mul(out=x6, in0=xt, scalar1=SCALE)
            y6 = singles.tile([128, 512], F32)
            nc.vector.tensor_scalar_mul(out=y6, in0=yt, scalar1=SCALE)
            x3 = singles.tile([128, 512], F32)
            nc.vector.tensor_scalar_mul(out=x3, in0=xt, scalar1=0.5 * SCALE)
            y3 = singles.tile([128, 512], F32)
            nc.vector.tensor_scalar_mul(out=y3, in0=yt, scalar1=0.5 * SCALE)

            M = psum.tile([128, 2, 512], F32)

            krb = krw.unsqueeze(1).broadcast_to([128, XG, KB])

            def make_g(side, v6, v3t, g):
                c0 = g * XG
                gt = gs.tile([128, XG, KB], F16, tag=f"g{side}")
                if PCHUNK[(side, g)]:
                    for i in range(XG):
                        nc.scalar.activation(
                            out=gt[:, i, :], in_=krw16h, func=tanh,
                            bias=v3t[:, c0 + i : c0 + i + 1], scale=1.0,
                        )
                    return gt
                arg = args.tile([128, XG, KB], F16, tag=f"arg{side}")
                nc.vector.tensor_tensor(
                    out=arg,
                    in0=v6[:, c0 : c0 + XG].unsqueeze(2).broadcast_to([128, XG, KB]),
                    in1=krb,
                    op=sub,
                )
                if SIG_PATH[(side, g)] == 'a':
                    nc.scalar.activation(out=gt, in_=arg, func=tanh, scale=0.5)
                else:
                    nc.vector._custom_dve(
                        op_q, out=gt[:, :, :], in0=arg[:, :, :],
                        s0=QC0, s1=QC1, imm2=QC2,
                    )
                return gt

            for g in range(NG):
                c0 = g * XG
                gx = make_g('x', x6, x3, g)
                gy = make_g('y', y6, y3, g)

                ph = phs.tile([128, XG, K], F16, tag="ph")
                eng = nc.gpsimd if DIFF_ENG[g] == 'g' else nc.vector
                eng.tensor_sub(
                    out=ph, in0=gx[:, :, 0:K], in1=gx[:, :, 1:KB]
                )

                for i in range(XG):
                    c = c0 + i
                    first = c == 0
                    last = c == NCHUNK - 1
                    for h in range(2):
                        nc.tensor.matmul(
                            M[:, h, 0:KB],
                            lhsT=ph[:, i, 128 * h : 128 * (h + 1)],
                            rhs=gy[:, i, :],
                            start=first,
                            stop=last,
                        )

            for h in range(2):
                t1 = work.tile([128, KB], F32, tag="ep")
                nc.scalar.activation(
                    out=t1,
                    in_=M[:, h, 0:KB],
                    func=mybir.ActivationFunctionType.Copy,
                    scale=INV,
                )
                t2 = work.tile([128, K], F32, tag="ep2")
                nc.vector.tensor_sub(out=t2, in0=t1[:, 0:K], in1=t1[:, 1:KB])
                nc.sync.dma_start(out=od[128 * h : 128 * (h + 1), :], in_=t2)

    nc.finalize()
    return nc


def _get_nc():
    global _cached_nc, _cache_key
    key = (tuple(sorted(SIG_PATH.items())), tuple(DIFF_ENG), tuple(sorted(PCHUNK.items())))
    if _cached_nc is None or _cache_key != key:
        _cached_nc = _build()
        _cache_key = key
    return _cached_nc


def _in_maps(x, y):
    x = np.ascontiguousarray(np.asarray(x, dtype=np.float32))
    y = np.ascontiguousarray(np.asarray(y, dtype=np.float32))
    krow = np.tile((2.5 * np.arange(KB, dtype=np.float32))[None, :], (128, 1))
    krw16h = np.tile((-1.25 * np.arange(KB, dtype=np.float32))[None, :], (128, 1)).astype(np.float16)
    return [
        {
            "x": x[b].reshape(128, 512),
            "y": y[b].reshape(128, 512),
            "krow": krow,
            "krw16h": krw16h,
        }
        for b in range(B)
    ]


def run(x, y, trace=False, **trace_kw):
    nc = _get_nc()
    res = run_bass_kernel_spmd(nc, _in_maps(x, y), list(range(B)), trace=trace,
                               **trace_kw)
    out = np.stack([res.results[b]["out"] for b in range(B)]).astype(np.float32)
    return out, res


def kernel(x, y):
    out, _ = run(x, y)
    return out


# revision 3
# speedup vs baseline: 1.0170x; 1.0144x over previous
"""Joint soft-histogram kernel v3 for Trainium2 (Bass/Tile), 8-core data parallel.

Math (per batch b, K=256, N=65536 pixels):
    g_j(v) = tanh((640v - 2.5j)/2) = 2*sigmoid(640v - 2.5j) - 1
    M[k,j] = sum_n (g_k - g_{k+1})(x_n) * g_j(y_n)   (PSUM, fp16 operands)
    out[k,j] = (M[k,j] - M[k,j+1]) / (4N)

Layout: pixels on partitions, i-major group tiles [128, XG=16, 257] so matmul
operands are contiguous (strided matmul APs measured 2.6-4.6x slower).
Two elementwise paths, split to balance DVE vs ScalarE (~290us each):
  - grouped: one fp32 DVE tensor_tensor builds arg = 640v - 2.5j via broadcast
    APs (fp16 2x is impossible here: the broadcast operand needs innermost
    stride 0, which disqualifies the 2x perf mode), then one big ScalarE ACT
    tanh(0.5*arg) -> fp16 (~3.6us per 4112-elem group).
  - per-chunk (SE only, no DVE): ACT tanh(1.0*krw16h + 320*v[p]) with the
    fp16-exact krow/2 table as input and a per-partition fp32 bias (~500ns per
    257-elem chunk) -- applied to ~15/32 of the y groups.
x side gets an fp16 2x-rate adjacent diff; 2 matmuls/chunk accumulate M in
PSUM over all 512 chunks. GPSIMD is left idle on purpose: its tensor ops run
at ~1.8ns/elem AND slow concurrent DVE by ~40% (port contention).

Sharding: pure data parallel, batch b -> core b.
"""

import numpy as np

import concourse.bass as bass
import concourse.tile as tile
from concourse import bacc, mybir
from concourse.bass_utils import run_bass_kernel_spmd

F32 = mybir.dt.float32
F16 = mybir.dt.float16

B = 8
K = 256
KB = K + 1
NPIX = 65536
NCHUNK = 512
XG = 16
NG = NCHUNK // XG
SCALE = 640.0
INV = 1.0 / (4.0 * NPIX)

QC0, QC1, QC2 = 0.46564883, -0.02206071, 0.00048341

# --- tuning knobs -----------------------------------------------------------
# per (side, group) sigma path: 'a' = ScalarE ACT tanh, 'q' = custom DVE quintic
SIG_PATH = {}
for g in range(NG):
    SIG_PATH[('x', g)] = 'a'
    SIG_PATH[('y', g)] = 'a'
# per-chunk ACT path (SE-only, no DVE expansion): dict (side, g) -> bool
PCHUNK = {}
for g in range(NG):
    PCHUNK[('y', g)] = (g % 2 == 0) and g != 30
    PCHUNK[('x', g)] = False
# engine for the x-side diff per group: 'v' (vector) or 'g' (gpsimd)
DIFF_ENG = ['v'] * NG
# ---------------------------------------------------------------------------

_cached_nc = None
_cache_key = None
_tanh_op = None


def _register_tanh_half_op():
    global _tanh_op
    if _tanh_op is not None:
        return _tanh_op
    import concourse.dve_ops as dvo
    from concourse.dve_spec import Spec, Src0, C0, C1, C2, Zero, One, sq, maxx, minn, lower
    from concourse.dve_uop import DveOpSpec

    NAME = "TANH_HALF_QUINT_ANT"
    for op in dvo.OPS:
        if op.name == NAME:
            _tanh_op = op
            return op

    t = sq(Src0)
    poly = Src0 * (C0 + t * (C1 + C2 * t))
    body = minn(maxx(poly, Zero - One), One)

    def _ref(in0, in1, s0, s1, imm2):
        tt = in0 * in0
        return np.clip(in0 * (s0 + tt * (s1 + imm2 * tt)), -1.0, 1.0)

    spec = Spec(body=body, reference=_ref)
    shas = {}
    for ver in ("v3", "v4"):
        uops = lower(spec, ver=ver)
        shas[ver] = DveOpSpec(name=NAME, opcode=1, uops=uops, rd1_en=False).sha(ver)
    op = dvo.DveOp(NAME, spec, subdim=False, uops_sha=shas)
    dvo.OPS.append(op)
    dvo._SUB_OPCODE_FOR_NAME[NAME] = dvo._CUSTOM_DVE_ROW_BASE + len(dvo.OPS) - 1
    assert dvo._SUB_OPCODE_FOR_NAME[NAME] < 0x20
    _tanh_op = op
    return op


def _build():
    op_q = _register_tanh_half_op()
    nc = bacc.Bacc("TRN2")
    xd = nc.declare_dram_parameter("x", [128, 512], F32, isOutput=False)
    yd = nc.declare_dram_parameter("y", [128, 512], F32, isOutput=False)
    kd = nc.declare_dram_parameter("krow", [128, KB], F32, isOutput=False)
    khd = nc.declare_dram_parameter("krw16h", [128, KB], F16, isOutput=False)
    od = nc.declare_dram_parameter("out", [256, 256], F32, isOutput=True)

    tanh = mybir.ActivationFunctionType.Tanh
    sub = mybir.AluOpType.subtract

    with tile.TileContext(nc) as tc:
        with (
            tc.tile_pool(name="singles", bufs=1) as singles,
            tc.tile_pool(name="args", bufs=4) as args,
            tc.tile_pool(name="gs", bufs=4) as gs,
            tc.tile_pool(name="phs", bufs=4) as phs,
            tc.tile_pool(name="work", bufs=3) as work,
            tc.tile_pool(name="psum", bufs=1, space="PSUM") as psum,
        ):
            xt = singles.tile([128, 512], F32)
            nc.sync.dma_start(out=xt, in_=xd[:, :])
            yt = singles.tile([128, 512], F32)
            nc.sync.dma_start(out=yt, in_=yd[:, :])
            krw = singles.tile([128, KB], F32)
            nc.sync.dma_start(out=krw, in_=kd[:, :])
            krw16h = singles.tile([128, KB], F16)
            nc.sync.dma_start(out=krw16h, in_=khd[:, :])

            x6 = singles.tile([128, 512], F32)
            nc.vector.tensor_scalar_mul(out=x6, in0=xt, scalar1=SCALE)
            y6 = singles.tile([128, 512], F32)
            nc.vector.tensor_scalar_mul(out=y6, in0=yt, scalar1=SCALE)
            x3 = y3 = None
            if any(PCHUNK[('x', g)] for g in range(NG)):
                x3 = singles.tile([128, 512], F32)
                nc.vector.tensor_scalar_mul(out=x3, in0=xt, scalar1=0.5 * SCALE)
            if any(PCHUNK[('y', g)] for g in range(NG)):
                y3 = singles.tile([128, 512], F32)
                nc.vector.tensor_scalar_mul(out=y3, in0=yt, scalar1=0.5 * SCALE)

            M = psum.tile([128, 2, 512], F32)

            krb = krw.unsqueeze(1).broadcast_to([128, XG, KB])

            def make_g(side, v6, v3t, g):
                c0 = g * XG
                gt = gs.tile([128, XG, KB], F16, tag=f"g{side}")
                if PCHUNK[(side, g)]:
                    for i in range(XG):
                        nc.scalar.activation(
                            out=gt[:, i, :], in_=krw16h, func=tanh,
                            bias=v3t[:, c0 + i : c0 + i + 1], scale=1.0,
                        )
                    return gt
                arg = args.tile([128, XG, KB], F16, tag=f"arg{side}")
                nc.vector.tensor_tensor(
                    out=arg,
                    in0=v6[:, c0 : c0 + XG].unsqueeze(2).broadcast_to([128, XG, KB]),
                    in1=krb,
                    op=sub,
                )
                if SIG_PATH[(side, g)] == 'a':
                    nc.scalar.activation(out=gt, in_=arg, func=tanh, scale=0.5)
                else:
                    nc.vector._custom_dve(
                        op_q, out=gt[:, :, :], in0=arg[:, :, :],
                        s0=QC0, s1=QC1, imm2=QC2,
                    )
                return gt

            for g in range(NG):
                c0 = g * XG
                gx = make_g('x', x6, x3, g)
                gy = make_g('y', y6, y3, g)

                ph = phs.tile([128, XG, K], F16, tag="ph")
                eng = nc.gpsimd if DIFF_ENG[g] == 'g' else nc.vector
                eng.tensor_sub(
                    out=ph, in0=gx[:, :, 0:K], in1=gx[:, :, 1:KB]
                )

                for i in range(XG):
                    c = c0 + i
                    first = c == 0
                    last = c == NCHUNK - 1
                    for h in range(2):
                        nc.tensor.matmul(
                            M[:, h, 0:KB],
                            lhsT=ph[:, i, 128 * h : 128 * (h + 1)],
                            rhs=gy[:, i, :],
                            start=first,
                            stop=last,
                        )

            for h in range(2):
                t1 = work.tile([128, KB], F32, tag="ep")
                nc.scalar.activation(
                    out=t1,
                    in_=M[:, h, 0:KB],
                    func=mybir.ActivationFunctionType.Copy,
                    scale=INV,
                )
                t2 = work.tile([128, K], F32, tag="ep2")
                nc.vector.tensor_sub(out=t2, in0=t1[:, 0:K], in1=t1[:, 1:KB])
                nc.sync.dma_start(out=od[128 * h : 128 * (h + 1), :], in_=t2)

    nc.finalize()
    return nc


def _get_nc():
    global _cached_nc, _cache_key
    key = (tuple(sorted(SIG_PATH.items())), tuple(DIFF_ENG), tuple(sorted(PCHUNK.items())))
    if _cached_nc is None or _cache_key != key:
        _cached_nc = _build()
        _cache_key = key
    return _cached_nc


def _in_maps(x, y):
    x = np.ascontiguousarray(np.asarray(x, dtype=np.float32))
    y = np.ascontiguousarray(np.asarray(y, dtype=np.float32))
    krow = np.tile((2.5 * np.arange(KB, dtype=np.float32))[None, :], (128, 1))
    krw16h = np.tile((-1.25 * np.arange(KB, dtype=np.float32))[None, :], (128, 1)).astype(np.float16)
    return [
        {
            "x": x[b].reshape(128, 512),
            "y": y[b].reshape(128, 512),
            "krow": krow,
            "krw16h": krw16h,
        }
        for b in range(B)
    ]


def run(x, y, trace=False, **trace_kw):
    nc = _get_nc()
    res = run_bass_kernel_spmd(nc, _in_maps(x, y), list(range(B)), trace=trace,
                               **trace_kw)
    out = np.stack([res.results[b]["out"] for b in range(B)]).astype(np.float32)
    return out, res


def kernel(x, y):
    out, _ = run(x, y)
    return out


# revision 4
# speedup vs baseline: 1.0226x; 1.0055x over previous
"""Joint soft-histogram kernel v3 for Trainium2 (Bass/Tile), 8-core data parallel.

Math (per batch b, K=256, N=65536 pixels):
    g_j(v) = tanh((640v - 2.5j)/2) = 2*sigmoid(640v - 2.5j) - 1
    M[k,j] = sum_n (g_k - g_{k+1})(x_n) * g_j(y_n)   (PSUM, fp16 operands)
    out[k,j] = (M[k,j] - M[k,j+1]) / (4N)

Layout: pixels on partitions, i-major group tiles [128, XG=16, 257] so matmul
operands are contiguous (strided matmul APs measured 2.6-4.6x slower).
Two elementwise paths, split to balance DVE vs ScalarE (~290us each):
  - grouped: one fp32 DVE tensor_tensor builds arg = 640v - 2.5j via broadcast
    APs (fp16 2x is impossible here: the broadcast operand needs innermost
    stride 0, which disqualifies the 2x perf mode), then one big ScalarE ACT
    tanh(0.5*arg) -> fp16 (~3.6us per 4112-elem group).
  - per-chunk (SE only, no DVE): ACT tanh(1.0*krw16h + 320*v[p]) with the
    fp16-exact krow/2 table as input and a per-partition fp32 bias (~500ns per
    257-elem chunk) -- applied to ~15/32 of the y groups.
x side gets an fp16 2x-rate adjacent diff; 2 matmuls/chunk accumulate M in
PSUM over all 512 chunks. GPSIMD is left idle on purpose: its tensor ops run
at ~1.8ns/elem AND slow concurrent DVE by ~40% (port contention).

Sharding: pure data parallel, batch b -> core b.
"""

import numpy as np

import concourse.bass as bass
import concourse.tile as tile
from concourse import bacc, mybir
from concourse.bass_utils import run_bass_kernel_spmd

F32 = mybir.dt.float32
F16 = mybir.dt.float16

B = 8
K = 256
KB = K + 1
NPIX = 65536
NCHUNK = 512
XG = 16
NG = NCHUNK // XG
SCALE = 640.0
INV = 1.0 / (4.0 * NPIX)

QC0, QC1, QC2 = 0.46564883, -0.02206071, 0.00048341

# --- tuning knobs -----------------------------------------------------------
# per (side, group) sigma path: 'a' = ScalarE ACT tanh, 'q' = custom DVE quintic
SIG_PATH = {}
for g in range(NG):
    SIG_PATH[('x', g)] = 'a'
    SIG_PATH[('y', g)] = 'a'
# per-chunk ACT path (SE-only, no DVE expansion): dict (side, g) -> bool
PCHUNK = {}
for g in range(NG):
    PCHUNK[('y', g)] = (g % 2 == 0) and g != 30
    PCHUNK[('x', g)] = g in (29, 31)
# engine for the x-side diff per group: 'v' (vector) or 'g' (gpsimd)
DIFF_ENG = ['v'] * NG
# ---------------------------------------------------------------------------

_cached_nc = None
_cache_key = None
_tanh_op = None


def _register_tanh_half_op():
    global _tanh_op
    if _tanh_op is not None:
        return _tanh_op
    import concourse.dve_ops as dvo
    from concourse.dve_spec import Spec, Src0, C0, C1, C2, Zero, One, sq, maxx, minn, lower
    from concourse.dve_uop import DveOpSpec

    NAME = "TANH_HALF_QUINT_ANT"
    for op in dvo.OPS:
        if op.name == NAME:
            _tanh_op = op
            return op

    t = sq(Src0)
    poly = Src0 * (C0 + t * (C1 + C2 * t))
    body = minn(maxx(poly, Zero - One), One)

    def _ref(in0, in1, s0, s1, imm2):
        tt = in0 * in0
        return np.clip(in0 * (s0 + tt * (s1 + imm2 * tt)), -1.0, 1.0)

    spec = Spec(body=body, reference=_ref)
    shas = {}
    for ver in ("v3", "v4"):
        uops = lower(spec, ver=ver)
        shas[ver] = DveOpSpec(name=NAME, opcode=1, uops=uops, rd1_en=False).sha(ver)
    op = dvo.DveOp(NAME, spec, subdim=False, uops_sha=shas)
    dvo.OPS.append(op)
    dvo._SUB_OPCODE_FOR_NAME[NAME] = dvo._CUSTOM_DVE_ROW_BASE + len(dvo.OPS) - 1
    assert dvo._SUB_OPCODE_FOR_NAME[NAME] < 0x20
    _tanh_op = op
    return op


def _build():
    op_q = _register_tanh_half_op()
    nc = bacc.Bacc("TRN2")
    xd = nc.declare_dram_parameter("x", [128, 512], F32, isOutput=False)
    yd = nc.declare_dram_parameter("y", [128, 512], F32, isOutput=False)
    kd = nc.declare_dram_parameter("krow", [128, KB], F32, isOutput=False)
    od = nc.declare_dram_parameter("out", [256, 256], F32, isOutput=True)

    tanh = mybir.ActivationFunctionType.Tanh
    sub = mybir.AluOpType.subtract

    with tile.TileContext(nc) as tc:
        with (
            tc.tile_pool(name="singles", bufs=1) as singles,
            tc.tile_pool(name="args", bufs=4) as args,
            tc.tile_pool(name="gs", bufs=4) as gs,
            tc.tile_pool(name="phs", bufs=4) as phs,
            tc.tile_pool(name="work", bufs=3) as work,
            tc.tile_pool(name="psum", bufs=1, space="PSUM") as psum,
        ):
            warm = singles.tile([128, 8], F16)
            nc.gpsimd.memset(warm, 0.25)
            warm2 = singles.tile([128, 8], F16)
            nc.scalar.activation(out=warm2, in_=warm, func=tanh)

            xt = singles.tile([128, 512], F32)
            nc.sync.dma_start(out=xt, in_=xd[:, :])
            yt = singles.tile([128, 512], F32)
            nc.scalar.dma_start(out=yt, in_=yd[:, :])
            krw = singles.tile([128, KB], F32)
            nc.sync.dma_start(out=krw, in_=kd[:, :])
            krw16h = singles.tile([128, KB], F16)
            nc.scalar.activation(
                out=krw16h, in_=krw,
                func=mybir.ActivationFunctionType.Copy, scale=-0.5,
            )

            x6 = singles.tile([128, 512], F32)
            nc.vector.tensor_scalar_mul(out=x6, in0=xt, scalar1=SCALE)
            x3 = y3 = None
            if any(PCHUNK[('y', g)] for g in range(NG)):
                y3 = singles.tile([128, 512], F32)
                nc.vector.tensor_scalar_mul(out=y3, in0=yt, scalar1=0.5 * SCALE)
            y6 = singles.tile([128, 512], F32)
            nc.vector.tensor_scalar_mul(out=y6, in0=yt, scalar1=SCALE)
            if any(PCHUNK[('x', g)] for g in range(NG)):
                x3 = singles.tile([128, 512], F32)
                nc.vector.tensor_scalar_mul(out=x3, in0=xt, scalar1=0.5 * SCALE)

            M = psum.tile([128, 2, 512], F32)

            krb = krw.unsqueeze(1).broadcast_to([128, XG, KB])

            def make_g(side, v6, v3t, g):
                c0 = g * XG
                gt = gs.tile([128, XG, KB], F16, tag=f"g{side}")
                if PCHUNK[(side, g)]:
                    for i in range(XG):
                        nc.scalar.activation(
                            out=gt[:, i, :], in_=krw16h, func=tanh,
                            bias=v3t[:, c0 + i : c0 + i + 1], scale=1.0,
                        )
                    return gt
                arg = args.tile([128, XG, KB], F16, tag=f"arg{side}")
                nc.vector.tensor_tensor(
                    out=arg,
                    in0=v6[:, c0 : c0 + XG].unsqueeze(2).broadcast_to([128, XG, KB]),
                    in1=krb,
                    op=sub,
                )
                if SIG_PATH[(side, g)] == 'a':
                    nc.scalar.activation(out=gt, in_=arg, func=tanh, scale=0.5)
                else:
                    nc.vector._custom_dve(
                        op_q, out=gt[:, :, :], in0=arg[:, :, :],
                        s0=QC0, s1=QC1, imm2=QC2,
                    )
                return gt

            for g in range(NG):
                c0 = g * XG
                if PCHUNK[('y', g)]:
                    gy = make_g('y', y6, y3, g)
                    gx = make_g('x', x6, x3, g)
                else:
                    gx = make_g('x', x6, x3, g)
                    gy = make_g('y', y6, y3, g)

                ph = phs.tile([128, XG, K], F16, tag="ph")
                eng = nc.gpsimd if DIFF_ENG[g] == 'g' else nc.vector
                eng.tensor_sub(
                    out=ph, in0=gx[:, :, 0:K], in1=gx[:, :, 1:KB]
                )

                for i in range(XG):
                    c = c0 + i
                    first = c == 0
                    last = c == NCHUNK - 1
                    for h in range(2):
                        nc.tensor.matmul(
                            M[:, h, 0:KB],
                            lhsT=ph[:, i, 128 * h : 128 * (h + 1)],
                            rhs=gy[:, i, :],
                            start=first,
                            stop=last,
                        )

            for h in range(2):
                t1 = work.tile([128, KB], F32, tag="ep")
                nc.scalar.activation(
                    out=t1,
                    in_=M[:, h, 0:KB],
                    func=mybir.ActivationFunctionType.Copy,
                    scale=INV,
                )
                t2 = work.tile([128, K], F32, tag="ep2")
                nc.vector.tensor_sub(out=t2, in0=t1[:, 0:K], in1=t1[:, 1:KB])
                nc.sync.dma_start(out=od[128 * h : 128 * (h + 1), :], in_=t2)

    nc.finalize()
    return nc


def _get_nc():
    global _cached_nc, _cache_key
    key = (tuple(sorted(SIG_PATH.items())), tuple(DIFF_ENG), tuple(sorted(PCHUNK.items())))
    if _cached_nc is None or _cache_key != key:
        _cached_nc = _build()
        _cache_key = key
    return _cached_nc


def _in_maps(x, y):
    x = np.ascontiguousarray(np.asarray(x, dtype=np.float32))
    y = np.ascontiguousarray(np.asarray(y, dtype=np.float32))
    krow = np.tile((2.5 * np.arange(KB, dtype=np.float32))[None, :], (128, 1))
    return [
        {
            "x": x[b].reshape(128, 512),
            "y": y[b].reshape(128, 512),
            "krow": krow,
        }
        for b in range(B)
    ]


def run(x, y, trace=False, **trace_kw):
    nc = _get_nc()
    res = run_bass_kernel_spmd(nc, _in_maps(x, y), list(range(B)), trace=trace,
                               **trace_kw)
    out = np.stack([res.results[b]["out"] for b in range(B)]).astype(np.float32)
    return out, res


def kernel(x, y):
    out, _ = run(x, y)
    return out


# revision 5
# speedup vs baseline: 1.0629x; 1.0394x over previous
"""Joint soft-histogram kernel v3 for Trainium2 (Bass/Tile), 8-core data parallel.

Math (per batch b, K=256, N=65536 pixels):
    g_j(v) = tanh((640v - 2.5j)/2) = 2*sigmoid(640v - 2.5j) - 1
    M[k,j] = sum_n (g_k - g_{k+1})(x_n) * g_j(y_n)   (PSUM, fp16 operands)
    out[k,j] = (M[k,j] - M[k,j+1]) / (4N)

Layout: pixels on partitions, i-major group tiles [128, XG=16, 257] so matmul
operands are contiguous (strided matmul APs measured 2.6-4.6x slower).
Two elementwise paths, split to balance DVE vs ScalarE (~290us each):
  - grouped: one fp32 DVE tensor_tensor builds arg = 640v - 2.5j via broadcast
    APs (fp16 2x is impossible here: the broadcast operand needs innermost
    stride 0, which disqualifies the 2x perf mode), then one big ScalarE ACT
    tanh(0.5*arg) -> fp16 (~3.6us per 4112-elem group).
  - per-chunk (SE only, no DVE): ACT tanh(1.0*krw16h + 320*v[p]) with the
    fp16-exact krow/2 table as input and a per-partition fp32 bias (~500ns per
    257-elem chunk) -- applied to ~15/32 of the y groups.
x side gets an fp16 2x-rate adjacent diff; 2 matmuls/chunk accumulate M in
PSUM over all 512 chunks. GPSIMD is left idle on purpose: its tensor ops run
at ~1.8ns/elem AND slow concurrent DVE by ~40% (port contention).

Sharding: pure data parallel, batch b -> core b.
"""

import numpy as np

import concourse.bass as bass
import concourse.tile as tile
from concourse import bacc, mybir
from concourse.bass_utils import run_bass_kernel_spmd

F32 = mybir.dt.float32
F16 = mybir.dt.float16

B = 8
K = 256
KB = K + 1
NPIX = 65536
NCHUNK = 512
XG = 16
NG = NCHUNK // XG
SCALE = 640.0
INV = 1.0 / (4.0 * NPIX)

QC0, QC1, QC2 = 0.46564883, -0.02206071, 0.00048341

# --- tuning knobs -----------------------------------------------------------
# per (side, group) sigma path: 'a' = ScalarE ACT tanh, 'q' = custom DVE quintic
SIG_PATH = {}
for g in range(NG):
    SIG_PATH[('x', g)] = 'a'
    SIG_PATH[('y', g)] = 'a'
# per-chunk ACT path (SE-only, no DVE expansion): dict (side, g) -> bool
PCHUNK = {}
for g in range(NG):
    PCHUNK[('y', g)] = (g % 2 == 0) and g != 30
    PCHUNK[('x', g)] = g in (29, 31)
# engine for the x-side diff per group: 'v' (vector) or 'g' (gpsimd)
DIFF_ENG = ['v'] * NG
# ---------------------------------------------------------------------------

_cached_nc = None
_cache_key = None
_tanh_op = None


def _register_tanh_half_op():
    global _tanh_op
    if _tanh_op is not None:
        return _tanh_op
    import concourse.dve_ops as dvo
    from concourse.dve_spec import Spec, Src0, C0, C1, C2, Zero, One, sq, maxx, minn, lower
    from concourse.dve_uop import DveOpSpec

    NAME = "TANH_HALF_QUINT_ANT"
    for op in dvo.OPS:
        if op.name == NAME:
            _tanh_op = op
            return op

    t = sq(Src0)
    poly = Src0 * (C0 + t * (C1 + C2 * t))
    body = minn(maxx(poly, Zero - One), One)

    def _ref(in0, in1, s0, s1, imm2):
        tt = in0 * in0
        return np.clip(in0 * (s0 + tt * (s1 + imm2 * tt)), -1.0, 1.0)

    spec = Spec(body=body, reference=_ref)
    shas = {}
    for ver in ("v3", "v4"):
        uops = lower(spec, ver=ver)
        shas[ver] = DveOpSpec(name=NAME, opcode=1, uops=uops, rd1_en=False).sha(ver)
    op = dvo.DveOp(NAME, spec, subdim=False, uops_sha=shas)
    dvo.OPS.append(op)
    dvo._SUB_OPCODE_FOR_NAME[NAME] = dvo._CUSTOM_DVE_ROW_BASE + len(dvo.OPS) - 1
    assert dvo._SUB_OPCODE_FOR_NAME[NAME] < 0x20
    _tanh_op = op
    return op


def _build():
    op_q = _register_tanh_half_op()
    nc = bacc.Bacc("TRN2")
    xd = nc.declare_dram_parameter("x", [128, 512], F32, isOutput=False)
    yd = nc.declare_dram_parameter("y", [128, 512], F32, isOutput=False)
    kd = nc.declare_dram_parameter("krow", [128, KB], F32, isOutput=False)
    od = nc.declare_dram_parameter("out", [256, 256], F32, isOutput=True)

    tanh = mybir.ActivationFunctionType.Tanh
    sub = mybir.AluOpType.subtract

    with tile.TileContext(nc) as tc:
        with (
            tc.tile_pool(name="singles", bufs=1) as singles,
            tc.tile_pool(name="args", bufs=4) as args,
            tc.tile_pool(name="gs", bufs=4) as gs,
            tc.tile_pool(name="phs", bufs=4) as phs,
            tc.tile_pool(name="work", bufs=3) as work,
            tc.tile_pool(name="psum", bufs=1, space="PSUM") as psum,
        ):
            warm = singles.tile([128, 8], F16)
            nc.gpsimd.memset(warm, 0.25)
            warm2 = singles.tile([128, 8], F16)
            nc.scalar.activation(out=warm2, in_=warm, func=tanh)

            xt = singles.tile([128, 512], F32)
            nc.sync.dma_start(out=xt, in_=xd[:, :])
            yt = singles.tile([128, 512], F32)
            nc.scalar.dma_start(out=yt, in_=yd[:, :])
            krw = singles.tile([128, KB], F32)
            nc.sync.dma_start(out=krw, in_=kd[:, :])
            krw16h = singles.tile([128, KB], F16)
            nc.scalar.activation(
                out=krw16h, in_=krw,
                func=mybir.ActivationFunctionType.Copy, scale=-0.5,
            )

            x6 = singles.tile([128, 512], F32)
            nc.vector.tensor_scalar_mul(out=x6, in0=xt, scalar1=SCALE)
            x3 = y3 = None
            if any(PCHUNK[('y', g)] for g in range(NG)):
                y3 = singles.tile([128, 512], F32)
                nc.vector.tensor_scalar_mul(out=y3, in0=yt, scalar1=0.5 * SCALE)
            y6 = singles.tile([128, 512], F32)
            nc.vector.tensor_scalar_mul(out=y6, in0=yt, scalar1=SCALE)
            if any(PCHUNK[('x', g)] for g in range(NG)):
                x3 = singles.tile([128, 512], F32)
                nc.vector.tensor_scalar_mul(out=x3, in0=xt, scalar1=0.5 * SCALE)

            M = psum.tile([128, 2, 512], F32)

            krb = krw.unsqueeze(1).broadcast_to([128, XG, KB])

            def make_g(side, v6, v3t, g):
                c0 = g * XG
                gt = gs.tile([128, XG, KB], F16, tag=f"g{side}")
                if PCHUNK[(side, g)]:
                    for i in range(XG):
                        nc.scalar.activation(
                            out=gt[:, i, :], in_=krw16h, func=tanh,
                            bias=v3t[:, c0 + i : c0 + i + 1], scale=1.0,
                        )
                    return gt
                arg = args.tile([128, XG, KB], F16, tag=f"arg{side}")
                nc.vector.tensor_tensor(
                    out=arg,
                    in0=v6[:, c0 : c0 + XG].unsqueeze(2).broadcast_to([128, XG, KB]),
                    in1=krb,
                    op=sub,
                )
                if SIG_PATH[(side, g)] == 'a':
                    nc.scalar.activation(out=gt, in_=arg, func=tanh, scale=0.5)
                else:
                    nc.vector._custom_dve(
                        op_q, out=gt[:, :, :], in0=arg[:, :, :],
                        s0=QC0, s1=QC1, imm2=QC2,
                    )
                return gt

            for g in range(NG):
                c0 = g * XG
                if PCHUNK[('y', g)] or PCHUNK[('x', g)]:
                    gy = make_g('y', y6, y3, g)
                    gx = make_g('x', x6, x3, g)
                else:
                    gx = make_g('x', x6, x3, g)
                    gy = make_g('y', y6, y3, g)

                ph = phs.tile([128, XG, K], F16, tag="ph")
                eng = nc.gpsimd if DIFF_ENG[g] == 'g' else nc.vector
                if PCHUNK[('x', g)]:
                    # gx comes from 16 serial per-chunk ACTs; split the diff so
                    # DVE overlaps the ACT burst instead of stalling on all 16
                    for q in range(4):
                        eng.tensor_sub(
                            out=ph[:, 4 * q : 4 * q + 4, :],
                            in0=gx[:, 4 * q : 4 * q + 4, 0:K],
                            in1=gx[:, 4 * q : 4 * q + 4, 1:KB],
                        )
                else:
                    eng.tensor_sub(
                        out=ph, in0=gx[:, :, 0:K], in1=gx[:, :, 1:KB]
                    )

                for i in range(XG):
                    c = c0 + i
                    first = c == 0
                    last = c == NCHUNK - 1
                    for h in range(2):
                        nc.tensor.matmul(
                            M[:, h, 0:KB],
                            lhsT=ph[:, i, 128 * h : 128 * (h + 1)],
                            rhs=gy[:, i, :],
                            start=first,
                            stop=last,
                        )

            for h in range(2):
                t1 = work.tile([128, KB], F32, tag="ep")
                nc.scalar.activation(
                    out=t1,
                    in_=M[:, h, 0:KB],
                    func=mybir.ActivationFunctionType.Copy,
                    scale=INV,
                )
                t2 = work.tile([128, K], F32, tag="ep2")
                nc.vector.tensor_sub(out=t2, in0=t1[:, 0:K], in1=t1[:, 1:KB])
                nc.sync.dma_start(out=od[128 * h : 128 * (h + 1), :], in_=t2)

    nc.finalize()
    return nc


def _get_nc():
    global _cached_nc, _cache_key
    key = (tuple(sorted(SIG_PATH.items())), tuple(DIFF_ENG), tuple(sorted(PCHUNK.items())))
    if _cached_nc is None or _cache_key != key:
        _cached_nc = _build()
        _cache_key = key
    return _cached_nc


def _in_maps(x, y):
    x = np.ascontiguousarray(np.asarray(x, dtype=np.float32))
    y = np.ascontiguousarray(np.asarray(y, dtype=np.float32))
    krow = np.tile((2.5 * np.arange(KB, dtype=np.float32))[None, :], (128, 1))
    return [
        {
            "x": x[b].reshape(128, 512),
            "y": y[b].reshape(128, 512),
            "krow": krow,
        }
        for b in range(B)
    ]


def run(x, y, trace=False, **trace_kw):
    nc = _get_nc()
    res = run_bass_kernel_spmd(nc, _in_maps(x, y), list(range(B)), trace=trace,
                               **trace_kw)
    out = np.stack([res.results[b]["out"] for b in range(B)]).astype(np.float32)
    return out, res


def kernel(x, y):
    out, _ = run(x, y)
    return out


# revision 6
# speedup vs baseline: 1.0652x; 1.0022x over previous
"""Joint soft-histogram kernel v3 for Trainium2 (Bass/Tile), 8-core data parallel.

Math (per batch b, K=256, N=65536 pixels):
    g_j(v) = tanh((640v - 2.5j)/2) = 2*sigmoid(640v - 2.5j) - 1
    M[k,j] = sum_n (g_k - g_{k+1})(x_n) * g_j(y_n)   (PSUM, fp16 operands)
    out[k,j] = (M[k,j] - M[k,j+1]) / (4N)

Layout: pixels on partitions, i-major group tiles [128, XG=16, 257] so matmul
operands are contiguous (strided matmul APs measured 2.6-4.6x slower).
Two elementwise paths, split to balance DVE vs ScalarE (~290us each):
  - grouped: one fp32 DVE tensor_tensor builds arg = 640v - 2.5j via broadcast
    APs (fp16 2x is impossible here: the broadcast operand needs innermost
    stride 0, which disqualifies the 2x perf mode), then one big ScalarE ACT
    tanh(0.5*arg) -> fp16 (~3.6us per 4112-elem group).
  - per-chunk (SE only, no DVE): ACT tanh(1.0*krw16h + 320*v[p]) with the
    fp16-exact krow/2 table as input and a per-partition fp32 bias (~500ns per
    257-elem chunk) -- applied to ~15/32 of the y groups.
x side gets an fp16 2x-rate adjacent diff; 2 matmuls/chunk accumulate M in
PSUM over all 512 chunks. GPSIMD is left idle on purpose: its tensor ops run
at ~1.8ns/elem AND slow concurrent DVE by ~40% (port contention).

Sharding: pure data parallel, batch b -> core b.
"""

import numpy as np

import concourse.bass as bass
import concourse.tile as tile
from concourse import bacc, mybir
from concourse.bass_utils import run_bass_kernel_spmd

F32 = mybir.dt.float32
F16 = mybir.dt.float16

B = 8
K = 256
KB = K + 1
NPIX = 65536
NCHUNK = 512
XG = 16
NG = NCHUNK // XG
SCALE = 640.0
INV = 1.0 / (4.0 * NPIX)

QC0, QC1, QC2 = 0.46564883, -0.02206071, 0.00048341

# --- tuning knobs -----------------------------------------------------------
# per (side, group) sigma path: 'a' = ScalarE ACT tanh, 'q' = custom DVE quintic
SIG_PATH = {}
for g in range(NG):
    SIG_PATH[('x', g)] = 'a'
    SIG_PATH[('y', g)] = 'a'
# per-chunk ACT path (SE-only, no DVE expansion): dict (side, g) -> bool
PCHUNK = {}
for g in range(NG):
    PCHUNK[('y', g)] = g in (0, 8, 16, 24)
    PCHUNK[('x', g)] = g in (29, 31)
# engine for the x-side diff per group: 'v' (vector) or 'g' (gpsimd)
DIFF_ENG = ['v'] * NG
# ---------------------------------------------------------------------------

_cached_nc = None
_cache_key = None
_tanh_op = None


def _register_tanh_half_op():
    global _tanh_op
    if _tanh_op is not None:
        return _tanh_op
    import concourse.dve_ops as dvo
    from concourse.dve_spec import Spec, Src0, C0, C1, C2, Zero, One, sq, maxx, minn, lower
    from concourse.dve_uop import DveOpSpec

    NAME = "TANH_HALF_QUINT_ANT"
    for op in dvo.OPS:
        if op.name == NAME:
            _tanh_op = op
            return op

    t = sq(Src0)
    poly = Src0 * (C0 + t * (C1 + C2 * t))
    body = minn(maxx(poly, Zero - One), One)

    def _ref(in0, in1, s0, s1, imm2):
        tt = in0 * in0
        return np.clip(in0 * (s0 + tt * (s1 + imm2 * tt)), -1.0, 1.0)

    spec = Spec(body=body, reference=_ref)
    shas = {}
    for ver in ("v3", "v4"):
        uops = lower(spec, ver=ver)
        shas[ver] = DveOpSpec(name=NAME, opcode=1, uops=uops, rd1_en=False).sha(ver)
    op = dvo.DveOp(NAME, spec, subdim=False, uops_sha=shas)
    dvo.OPS.append(op)
    dvo._SUB_OPCODE_FOR_NAME[NAME] = dvo._CUSTOM_DVE_ROW_BASE + len(dvo.OPS) - 1
    assert dvo._SUB_OPCODE_FOR_NAME[NAME] < 0x20
    _tanh_op = op
    return op


def _build():
    op_q = _register_tanh_half_op()
    nc = bacc.Bacc("TRN2")
    xd = nc.declare_dram_parameter("x", [128, 512], F32, isOutput=False)
    yd = nc.declare_dram_parameter("y", [128, 512], F32, isOutput=False)
    kd = nc.declare_dram_parameter("krow", [128, KB], F32, isOutput=False)
    od = nc.declare_dram_parameter("out", [256, 256], F32, isOutput=True)

    tanh = mybir.ActivationFunctionType.Tanh
    sub = mybir.AluOpType.subtract

    with tile.TileContext(nc) as tc:
        with (
            tc.tile_pool(name="singles", bufs=1) as singles,
            tc.tile_pool(name="args", bufs=4) as args,
            tc.tile_pool(name="gs", bufs=4) as gs,
            tc.tile_pool(name="work", bufs=3) as work,
            tc.tile_pool(name="psum", bufs=1, space="PSUM") as psum,
        ):
            warm = singles.tile([128, 8], F16)
            nc.gpsimd.memset(warm, 0.25)
            warm2 = singles.tile([128, 8], F16)
            nc.scalar.activation(out=warm2, in_=warm, func=tanh)

            xt = singles.tile([128, 512], F32)
            nc.sync.dma_start(out=xt, in_=xd[:, :])
            yt = singles.tile([128, 512], F32)
            nc.scalar.dma_start(out=yt, in_=yd[:, :])
            krw = singles.tile([128, KB], F32)
            nc.sync.dma_start(out=krw, in_=kd[:, :])
            krw16h = singles.tile([128, KB], F16)
            nc.scalar.activation(
                out=krw16h, in_=krw,
                func=mybir.ActivationFunctionType.Copy, scale=-0.5,
            )

            x6 = singles.tile([128, 512], F32)
            nc.vector.tensor_scalar_mul(out=x6, in0=xt, scalar1=SCALE)
            x3 = y3 = None
            if any(PCHUNK[('y', g)] for g in range(NG)):
                y3 = singles.tile([128, 512], F32)
                nc.vector.tensor_scalar_mul(out=y3, in0=yt, scalar1=0.5 * SCALE)
            y6 = singles.tile([128, 512], F32)
            nc.vector.tensor_scalar_mul(out=y6, in0=yt, scalar1=SCALE)
            if any(PCHUNK[('x', g)] for g in range(NG)):
                x3 = singles.tile([128, 512], F32)
                nc.vector.tensor_scalar_mul(out=x3, in0=xt, scalar1=0.5 * SCALE)

            M = psum.tile([128, 2, 512], F32)
            M2 = psum.tile([128, 2, 512], F32)

            krb = krw.unsqueeze(1).broadcast_to([128, XG, KB])

            def make_g(side, v6, v3t, g):
                c0 = g * XG
                gt = gs.tile([128, XG, KB], F16, tag=f"g{side}")
                if PCHUNK[(side, g)]:
                    for i in range(XG):
                        nc.scalar.activation(
                            out=gt[:, i, :], in_=krw16h, func=tanh,
                            bias=v3t[:, c0 + i : c0 + i + 1], scale=1.0,
                        )
                    return gt
                arg = args.tile([128, XG, KB], F16, tag=f"arg{side}")
                nc.vector.tensor_tensor(
                    out=arg,
                    in0=v6[:, c0 : c0 + XG].unsqueeze(2).broadcast_to([128, XG, KB]),
                    in1=krb,
                    op=sub,
                )
                if SIG_PATH[(side, g)] == 'a':
                    nc.scalar.activation(out=gt, in_=arg, func=tanh, scale=0.5)
                else:
                    nc.vector._custom_dve(
                        op_q, out=gt[:, :, :], in0=arg[:, :, :],
                        s0=QC0, s1=QC1, imm2=QC2,
                    )
                return gt

            for g in range(NG):
                c0 = g * XG
                if PCHUNK[('y', g)] or PCHUNK[('x', g)]:
                    gy = make_g('y', y6, y3, g)
                    gx = make_g('x', x6, x3, g)
                else:
                    gx = make_g('x', x6, x3, g)
                    gy = make_g('y', y6, y3, g)

                for i in range(XG):
                    c = c0 + i
                    first = c == 0
                    last = c == NCHUNK - 1
                    for h in range(2):
                        nc.tensor.matmul(
                            M[:, h, 0:KB],
                            lhsT=gx[:, i, 128 * h : 128 * h + 128],
                            rhs=gy[:, i, :],
                            start=first,
                            stop=last,
                        )
                        nc.tensor.matmul(
                            M2[:, h, 0:KB],
                            lhsT=gx[:, i, 128 * h + 1 : 128 * h + 129],
                            rhs=gy[:, i, :],
                            start=first,
                            stop=last,
                        )

            for h in range(2):
                m1s = work.tile([128, KB], F32, tag="ep1")
                nc.scalar.activation(
                    out=m1s, in_=M[:, h, 0:KB],
                    func=mybir.ActivationFunctionType.Copy, scale=INV,
                )
                m2s = work.tile([128, KB], F32, tag="ep2")
                nc.scalar.activation(
                    out=m2s, in_=M2[:, h, 0:KB],
                    func=mybir.ActivationFunctionType.Copy, scale=INV,
                )
                t2 = work.tile([128, KB], F32, tag="ep3")
                nc.vector.tensor_sub(out=t2, in0=m1s, in1=m2s)
                t3 = work.tile([128, K], F32, tag="ep4")
                nc.vector.tensor_sub(out=t3, in0=t2[:, 0:K], in1=t2[:, 1:KB])
                nc.sync.dma_start(out=od[128 * h : 128 * (h + 1), :], in_=t3)

    nc.finalize()
    return nc


def _get_nc():
    global _cached_nc, _cache_key
    key = (tuple(sorted(SIG_PATH.items())), tuple(DIFF_ENG), tuple(sorted(PCHUNK.items())))
    if _cached_nc is None or _cache_key != key:
        _cached_nc = _build()
        _cache_key = key
    return _cached_nc


def _in_maps(x, y):
    x = np.ascontiguousarray(np.asarray(x, dtype=np.float32))
    y = np.ascontiguousarray(np.asarray(y, dtype=np.float32))
    krow = np.tile((2.5 * np.arange(KB, dtype=np.float32))[None, :], (128, 1))
    return [
        {
            "x": x[b].reshape(128, 512),
            "y": y[b].reshape(128, 512),
            "krow": krow,
        }
        for b in range(B)
    ]


def run(x, y, trace=False, **trace_kw):
    nc = _get_nc()
    res = run_bass_kernel_spmd(nc, _in_maps(x, y), list(range(B)), trace=trace,
                               **trace_kw)
    out = np.stack([res.results[b]["out"] for b in range(B)]).astype(np.float32)
    return out, res


def kernel(x, y):
    out, _ = run(x, y)
    return out


# revision 7
# speedup vs baseline: 1.0822x; 1.0160x over previous
"""Joint soft-histogram kernel v3 for Trainium2 (Bass/Tile), 8-core data parallel.

Math (per batch b, K=256, N=65536 pixels):
    g_j(v) = tanh((640v - 2.5j)/2) = 2*sigmoid(640v - 2.5j) - 1
    M1[k,j] = sum_n g_k(x_n) g_j(y_n),  M2[k,j] = sum_n g_{k+1}(x_n) g_j(y_n)
    out[k,j] = ((M1-M2)[k,j] - (M1-M2)[k,j+1]) / (4N)

Diff-free double accumulation: the x-side adjacent diff (74us of DVE) is
replaced by a second PSUM accumulation whose lhsT is the SAME gx tile sliced
one bin over -- zero extra elementwise work, 4 matmuls/chunk instead of 2.
The busy TensorE warms the PE HAM clock gate to 2.4 GHz, so 2048 matmuls cost
~237us, landing all three engines balanced at ~256/256/237us.

Layout: pixels on partitions, i-major group tiles [128, XG=16, 257] so matmul
operands are contiguous (strided matmul APs measured 2.6-4.6x slower).
Two elementwise paths, split to balance DVE vs ScalarE:
  - grouped: one fp32 DVE tensor_tensor builds arg = 640v - 2.5j via broadcast
    APs (fp16 2x is impossible: the broadcast operand needs innermost stride 0,
    which disqualifies the 2x perf mode), then one big ScalarE ACT
    tanh(0.5*arg) -> fp16 (~3.6us per 4112-elem group).
  - per-chunk (SE only, no DVE): ACT tanh(1.0*krw16h + 320*v[p]) with the
    fp16-exact krow/2 table as input and a per-partition fp32 bias (~500ns per
    257-elem chunk) -- 4 y groups + the 2 tail x groups.
GPSIMD is left idle on purpose: its tensor ops run at ~1.8ns/elem AND slow
concurrent DVE by ~40% (port contention).

Sharding: pure data parallel, batch b -> core b.
"""

import numpy as np

import concourse.bass as bass
import concourse.tile as tile
from concourse import bacc, mybir
from concourse.bass_utils import run_bass_kernel_spmd

F32 = mybir.dt.float32
F16 = mybir.dt.float16

B = 8
K = 256
KB = K + 1
NPIX = 65536
NCHUNK = 512
XG = 16
NG = NCHUNK // XG
SCALE = 640.0
INV = 1.0 / (4.0 * NPIX)

QC0, QC1, QC2 = 0.46564883, -0.02206071, 0.00048341

# --- tuning knobs -----------------------------------------------------------
# per (side, group) sigma path: 'a' = ScalarE ACT tanh, 'q' = custom DVE quintic
SIG_PATH = {}
for g in range(NG):
    SIG_PATH[('x', g)] = 'a'
    SIG_PATH[('y', g)] = 'a'
# per-chunk ACT path (SE-only, no DVE expansion): dict (side, g) -> bool
PCHUNK = {}
for g in range(NG):
    PCHUNK[('y', g)] = g in (0, 8, 16, 24)
    PCHUNK[('x', g)] = g in (29, 31)
# engine for the x-side diff per group: 'v' (vector) or 'g' (gpsimd)
DIFF_ENG = ['v'] * NG
# ---------------------------------------------------------------------------

_cached_nc = None
_cache_key = None
_tanh_op = None


def _register_tanh_half_op():
    global _tanh_op
    if _tanh_op is not None:
        return _tanh_op
    import concourse.dve_ops as dvo
    from concourse.dve_spec import Spec, Src0, C0, C1, C2, Zero, One, sq, maxx, minn, lower
    from concourse.dve_uop import DveOpSpec

    NAME = "TANH_HALF_QUINT_ANT"
    for op in dvo.OPS:
        if op.name == NAME:
            _tanh_op = op
            return op

    t = sq(Src0)
    poly = Src0 * (C0 + t * (C1 + C2 * t))
    body = minn(maxx(poly, Zero - One), One)

    def _ref(in0, in1, s0, s1, imm2):
        tt = in0 * in0
        return np.clip(in0 * (s0 + tt * (s1 + imm2 * tt)), -1.0, 1.0)

    spec = Spec(body=body, reference=_ref)
    shas = {}
    for ver in ("v3", "v4"):
        uops = lower(spec, ver=ver)
        shas[ver] = DveOpSpec(name=NAME, opcode=1, uops=uops, rd1_en=False).sha(ver)
    op = dvo.DveOp(NAME, spec, subdim=False, uops_sha=shas)
    dvo.OPS.append(op)
    dvo._SUB_OPCODE_FOR_NAME[NAME] = dvo._CUSTOM_DVE_ROW_BASE + len(dvo.OPS) - 1
    assert dvo._SUB_OPCODE_FOR_NAME[NAME] < 0x20
    _tanh_op = op
    return op


def _build():
    op_q = _register_tanh_half_op()
    nc = bacc.Bacc("TRN2")
    xd = nc.declare_dram_parameter("x", [128, 512], F32, isOutput=False)
    yd = nc.declare_dram_parameter("y", [128, 512], F32, isOutput=False)
    kd = nc.declare_dram_parameter("krow", [128, KB], F32, isOutput=False)
    od = nc.declare_dram_parameter("out", [256, 256], F32, isOutput=True)

    tanh = mybir.ActivationFunctionType.Tanh
    sub = mybir.AluOpType.subtract

    with tile.TileContext(nc) as tc:
        with (
            tc.tile_pool(name="singles", bufs=1) as singles,
            tc.tile_pool(name="args", bufs=4) as args,
            tc.tile_pool(name="gs", bufs=4) as gs,
            tc.tile_pool(name="work", bufs=3) as work,
            tc.tile_pool(name="psum", bufs=1, space="PSUM") as psum,
        ):
            warm = singles.tile([128, 8], F16)
            nc.gpsimd.memset(warm, 0.25)
            warm2 = singles.tile([128, 8], F16)
            nc.scalar.activation(out=warm2, in_=warm, func=tanh)

            xt = singles.tile([128, 512], F32)
            nc.sync.dma_start(out=xt, in_=xd[:, :])
            yt = singles.tile([128, 512], F32)
            nc.scalar.dma_start(out=yt, in_=yd[:, :])
            krw = singles.tile([128, KB], F32)
            nc.sync.dma_start(out=krw, in_=kd[:, :])
            krw16h = singles.tile([128, KB], F16)
            nc.scalar.activation(
                out=krw16h, in_=krw,
                func=mybir.ActivationFunctionType.Copy, scale=-0.5,
            )

            x6 = singles.tile([128, 512], F32)
            nc.vector.tensor_scalar_mul(out=x6, in0=xt, scalar1=SCALE)
            x3 = y3 = None
            if any(PCHUNK[('y', g)] for g in range(NG)):
                y3 = singles.tile([128, 512], F32)
                nc.vector.tensor_scalar_mul(out=y3, in0=yt, scalar1=0.5 * SCALE)
            y6 = singles.tile([128, 512], F32)
            nc.vector.tensor_scalar_mul(out=y6, in0=yt, scalar1=SCALE)
            if any(PCHUNK[('x', g)] for g in range(NG)):
                x3 = singles.tile([128, 512], F32)
                nc.vector.tensor_scalar_mul(out=x3, in0=xt, scalar1=0.5 * SCALE)

            M = psum.tile([128, 2, 512], F32)
            M2 = psum.tile([128, 2, 512], F32)

            krb = krw.unsqueeze(1).broadcast_to([128, XG, KB])

            def make_g(side, v6, v3t, g):
                c0 = g * XG
                gt = gs.tile([128, XG, KB], F16, tag=f"g{side}")
                if PCHUNK[(side, g)]:
                    for i in range(XG):
                        nc.scalar.activation(
                            out=gt[:, i, :], in_=krw16h, func=tanh,
                            bias=v3t[:, c0 + i : c0 + i + 1], scale=1.0,
                        )
                    return gt
                arg = args.tile([128, XG, KB], F16, tag=f"arg{side}")
                nc.vector.tensor_tensor(
                    out=arg,
                    in0=v6[:, c0 : c0 + XG].unsqueeze(2).broadcast_to([128, XG, KB]),
                    in1=krb,
                    op=sub,
                )
                if SIG_PATH[(side, g)] == 'a':
                    nc.scalar.activation(out=gt, in_=arg, func=tanh, scale=0.5)
                else:
                    nc.vector._custom_dve(
                        op_q, out=gt[:, :, :], in0=arg[:, :, :],
                        s0=QC0, s1=QC1, imm2=QC2,
                    )
                return gt

            for g in range(NG):
                c0 = g * XG
                if PCHUNK[('y', g)] or PCHUNK[('x', g)]:
                    gy = make_g('y', y6, y3, g)
                    gx = make_g('x', x6, x3, g)
                else:
                    gx = make_g('x', x6, x3, g)
                    gy = make_g('y', y6, y3, g)

                for i in range(XG):
                    c = c0 + i
                    first = c == 0
                    last = c == NCHUNK - 1
                    for h in range(2):
                        nc.tensor.matmul(
                            M[:, h, 0:KB],
                            lhsT=gx[:, i, 128 * h : 128 * h + 128],
                            rhs=gy[:, i, :],
                            start=first,
                            stop=last,
                        )
                        nc.tensor.matmul(
                            M2[:, h, 0:KB],
                            lhsT=gx[:, i, 128 * h + 1 : 128 * h + 129],
                            rhs=gy[:, i, :],
                            start=first,
                            stop=last,
                        )

            for h in range(2):
                m1s = work.tile([128, KB], F32, tag="ep1")
                nc.scalar.activation(
                    out=m1s, in_=M[:, h, 0:KB],
                    func=mybir.ActivationFunctionType.Copy, scale=INV,
                )
                m2s = work.tile([128, KB], F32, tag="ep2")
                nc.scalar.activation(
                    out=m2s, in_=M2[:, h, 0:KB],
                    func=mybir.ActivationFunctionType.Copy, scale=INV,
                )
                t2 = work.tile([128, KB], F32, tag="ep3")
                nc.vector.tensor_sub(out=t2, in0=m1s, in1=m2s)
                t3 = work.tile([128, K], F32, tag="ep4")
                nc.vector.tensor_sub(out=t3, in0=t2[:, 0:K], in1=t2[:, 1:KB])
                nc.sync.dma_start(out=od[128 * h : 128 * (h + 1), :], in_=t3)

    nc.finalize()
    return nc


def _get_nc():
    global _cached_nc, _cache_key
    key = (tuple(sorted(SIG_PATH.items())), tuple(DIFF_ENG), tuple(sorted(PCHUNK.items())))
    if _cached_nc is None or _cache_key != key:
        _cached_nc = _build()
        _cache_key = key
    return _cached_nc


def _in_maps(x, y):
    x = np.ascontiguousarray(np.asarray(x, dtype=np.float32))
    y = np.ascontiguousarray(np.asarray(y, dtype=np.float32))
    krow = np.tile((2.5 * np.arange(KB, dtype=np.float32))[None, :], (128, 1))
    return [
        {
            "x": x[b].reshape(128, 512),
            "y": y[b].reshape(128, 512),
            "krow": krow,
        }
        for b in range(B)
    ]


def run(x, y, trace=False, **trace_kw):
    nc = _get_nc()
    res = run_bass_kernel_spmd(nc, _in_maps(x, y), list(range(B)), trace=trace,
                               **trace_kw)
    out = np.stack([res.results[b]["out"] for b in range(B)]).astype(np.float32)
    return out, res


def kernel(x, y):
    out, _ = run(x, y)
    return out


# revision 8
# speedup vs baseline: 1.0955x; 1.0122x over previous
"""Joint soft-histogram kernel v3 for Trainium2 (Bass/Tile), 8-core data parallel.

Math (per batch b, K=256, N=65536 pixels):
    g_j(v) = tanh((640v - 2.5j)/2) = 2*sigmoid(640v - 2.5j) - 1
    M1[k,j] = sum_n g_k(x_n) g_j(y_n),  M2[k,j] = sum_n g_{k+1}(x_n) g_j(y_n)
    out[k,j] = ((M1-M2)[k,j] - (M1-M2)[k,j+1]) / (4N)

Diff-free double accumulation: the x-side adjacent diff (74us of DVE) is
replaced by a second PSUM accumulation whose lhsT is the SAME gx tile sliced
one bin over -- zero extra elementwise work, 4 matmuls/chunk instead of 2.
The busy TensorE warms the PE HAM clock gate to 2.4 GHz, so 2048 matmuls cost
~237us, landing all three engines balanced at ~256/256/237us.

Layout: pixels on partitions, i-major group tiles [128, XG=16, 257] so matmul
operands are contiguous (strided matmul APs measured 2.6-4.6x slower).
Two elementwise paths, split to balance DVE vs ScalarE:
  - grouped: one fp32 DVE tensor_tensor builds arg = 640v - 2.5j via broadcast
    APs (fp16 2x is impossible: the broadcast operand needs innermost stride 0,
    which disqualifies the 2x perf mode), then one big ScalarE ACT
    tanh(0.5*arg) -> fp16 (~3.6us per 4112-elem group).
  - per-chunk (SE only, no DVE): ACT tanh(1.0*krw16h + 320*v[p]) with the
    fp16-exact krow/2 table as input and a per-partition fp32 bias (~500ns per
    257-elem chunk) -- 4 y groups + the 2 tail x groups.
GPSIMD is left idle on purpose: its tensor ops run at ~1.8ns/elem AND slow
concurrent DVE by ~40% (port contention).

Sharding: pure data parallel, batch b -> core b.
"""

import numpy as np

import concourse.bass as bass
import concourse.tile as tile
from concourse import bacc, mybir
from concourse.bass_utils import run_bass_kernel_spmd

F32 = mybir.dt.float32
F16 = mybir.dt.float16

B = 8
K = 256
KB = K + 1
NPIX = 65536
NCHUNK = 512
XG = 16
NG = NCHUNK // XG
SCALE = 640.0
INV = 1.0 / (4.0 * NPIX)

QC0, QC1, QC2 = 0.46564883, -0.02206071, 0.00048341

# --- tuning knobs -----------------------------------------------------------
# per (side, group) sigma path: 'a' = ScalarE ACT tanh, 'q' = custom DVE quintic
SIG_PATH = {}
for g in range(NG):
    SIG_PATH[('x', g)] = 'a'
    SIG_PATH[('y', g)] = 'a'
# per-chunk ACT path (SE-only, no DVE expansion): dict (side, g) -> bool
PCHUNK = {}
for g in range(NG):
    PCHUNK[('y', g)] = g in (2, 7, 12, 17, 22, 27)
    PCHUNK[('x', g)] = g in (29, 31)
# engine for the x-side diff per group: 'v' (vector) or 'g' (gpsimd)
DIFF_ENG = ['v'] * NG
# ---------------------------------------------------------------------------

_cached_nc = None
_cache_key = None
_tanh_op = None


def _register_tanh_half_op():
    global _tanh_op
    if _tanh_op is not None:
        return _tanh_op
    import concourse.dve_ops as dvo
    from concourse.dve_spec import Spec, Src0, C0, C1, C2, Zero, One, sq, maxx, minn, lower
    from concourse.dve_uop import DveOpSpec

    NAME = "TANH_HALF_QUINT_ANT"
    for op in dvo.OPS:
        if op.name == NAME:
            _tanh_op = op
            return op

    t = sq(Src0)
    poly = Src0 * (C0 + t * (C1 + C2 * t))
    body = minn(maxx(poly, Zero - One), One)

    def _ref(in0, in1, s0, s1, imm2):
        tt = in0 * in0
        return np.clip(in0 * (s0 + tt * (s1 + imm2 * tt)), -1.0, 1.0)

    spec = Spec(body=body, reference=_ref)
    shas = {}
    for ver in ("v3", "v4"):
        uops = lower(spec, ver=ver)
        shas[ver] = DveOpSpec(name=NAME, opcode=1, uops=uops, rd1_en=False).sha(ver)
    op = dvo.DveOp(NAME, spec, subdim=False, uops_sha=shas)
    dvo.OPS.append(op)
    dvo._SUB_OPCODE_FOR_NAME[NAME] = dvo._CUSTOM_DVE_ROW_BASE + len(dvo.OPS) - 1
    assert dvo._SUB_OPCODE_FOR_NAME[NAME] < 0x20
    _tanh_op = op
    return op


def _build():
    op_q = _register_tanh_half_op()
    nc = bacc.Bacc("TRN2")
    xd = nc.declare_dram_parameter("x", [128, 512], F32, isOutput=False)
    yd = nc.declare_dram_parameter("y", [128, 512], F32, isOutput=False)
    kd = nc.declare_dram_parameter("krow", [128, KB], F32, isOutput=False)
    od = nc.declare_dram_parameter("out", [256, 256], F32, isOutput=True)

    tanh = mybir.ActivationFunctionType.Tanh
    sub = mybir.AluOpType.subtract

    with tile.TileContext(nc) as tc:
        with (
            tc.tile_pool(name="singles", bufs=1) as singles,
            tc.tile_pool(name="args", bufs=4) as args,
            tc.tile_pool(name="gs", bufs=6) as gs,
            tc.tile_pool(name="work", bufs=3) as work,
            tc.tile_pool(name="psum", bufs=1, space="PSUM") as psum,
        ):
            warm = singles.tile([128, 8], F16)
            nc.gpsimd.memset(warm, 0.25)
            warm2 = singles.tile([128, 8], F16)
            nc.scalar.activation(out=warm2, in_=warm, func=tanh)

            xt = singles.tile([128, 512], F32)
            nc.sync.dma_start(out=xt, in_=xd[:, :])
            yt = singles.tile([128, 512], F32)
            nc.scalar.dma_start(out=yt, in_=yd[:, :])
            krw = singles.tile([128, KB], F32)
            nc.sync.dma_start(out=krw, in_=kd[:, :])
            krw16h = singles.tile([128, KB], F16)
            nc.scalar.activation(
                out=krw16h, in_=krw,
                func=mybir.ActivationFunctionType.Copy, scale=-0.5,
            )

            x6 = singles.tile([128, 512], F32)
            nc.vector.tensor_scalar_mul(out=x6, in0=xt, scalar1=SCALE)
            x3 = y3 = None
            if any(PCHUNK[('y', g)] for g in range(NG)):
                y3 = singles.tile([128, 512], F32)
                nc.vector.tensor_scalar_mul(out=y3, in0=yt, scalar1=0.5 * SCALE)
            y6 = singles.tile([128, 512], F32)
            nc.vector.tensor_scalar_mul(out=y6, in0=yt, scalar1=SCALE)
            if any(PCHUNK[('x', g)] for g in range(NG)):
                x3 = singles.tile([128, 512], F32)
                nc.vector.tensor_scalar_mul(out=x3, in0=xt, scalar1=0.5 * SCALE)

            M = psum.tile([128, 2, 512], F32)
            M2 = psum.tile([128, 2, 512], F32)

            krb = krw.unsqueeze(1).broadcast_to([128, XG, KB])

            def make_g(side, v6, v3t, g):
                c0 = g * XG
                gt = gs.tile([128, XG, KB], F16, tag=f"g{side}")
                if PCHUNK[(side, g)]:
                    for i in range(XG):
                        nc.scalar.activation(
                            out=gt[:, i, :], in_=krw16h, func=tanh,
                            bias=v3t[:, c0 + i : c0 + i + 1], scale=1.0,
                        )
                    return gt
                arg = args.tile([128, XG, KB], F16, tag=f"arg{side}")
                nc.vector.tensor_tensor(
                    out=arg,
                    in0=v6[:, c0 : c0 + XG].unsqueeze(2).broadcast_to([128, XG, KB]),
                    in1=krb,
                    op=sub,
                )
                if SIG_PATH[(side, g)] == 'a':
                    nc.scalar.activation(out=gt, in_=arg, func=tanh, scale=0.5)
                else:
                    nc.vector._custom_dve(
                        op_q, out=gt[:, :, :], in0=arg[:, :, :],
                        s0=QC0, s1=QC1, imm2=QC2,
                    )
                return gt

            for g in range(NG):
                c0 = g * XG
                if PCHUNK[('y', g)] or PCHUNK[('x', g)]:
                    gy = make_g('y', y6, y3, g)
                    gx = make_g('x', x6, x3, g)
                else:
                    gx = make_g('x', x6, x3, g)
                    gy = make_g('y', y6, y3, g)

                for i in range(XG):
                    c = c0 + i
                    first = c == 0
                    last = c == NCHUNK - 1
                    for h in range(2):
                        nc.tensor.matmul(
                            M[:, h, 0:KB],
                            lhsT=gx[:, i, 128 * h : 128 * h + 128],
                            rhs=gy[:, i, :],
                            start=first,
                            stop=last,
                        )
                        nc.tensor.matmul(
                            M2[:, h, 0:KB],
                            lhsT=gx[:, i, 128 * h + 1 : 128 * h + 129],
                            rhs=gy[:, i, :],
                            start=first,
                            stop=last,
                        )

            for h in range(2):
                m1s = work.tile([128, KB], F32, tag="ep1")
                nc.scalar.activation(
                    out=m1s, in_=M[:, h, 0:KB],
                    func=mybir.ActivationFunctionType.Copy, scale=INV,
                )
                m2s = work.tile([128, KB], F32, tag="ep2")
                nc.scalar.activation(
                    out=m2s, in_=M2[:, h, 0:KB],
                    func=mybir.ActivationFunctionType.Copy, scale=INV,
                )
                t2 = work.tile([128, KB], F32, tag="ep3")
                nc.vector.tensor_sub(out=t2, in0=m1s, in1=m2s)
                t3 = work.tile([128, K], F32, tag="ep4")
                nc.vector.tensor_sub(out=t3, in0=t2[:, 0:K], in1=t2[:, 1:KB])
                nc.sync.dma_start(out=od[128 * h : 128 * (h + 1), :], in_=t3)

    nc.finalize()
    return nc


def _get_nc():
    global _cached_nc, _cache_key
    key = (tuple(sorted(SIG_PATH.items())), tuple(DIFF_ENG), tuple(sorted(PCHUNK.items())))
    if _cached_nc is None or _cache_key != key:
        _cached_nc = _build()
        _cache_key = key
    return _cached_nc


def _in_maps(x, y):
    x = np.ascontiguousarray(np.asarray(x, dtype=np.float32))
    y = np.ascontiguousarray(np.asarray(y, dtype=np.float32))
    krow = np.tile((2.5 * np.arange(KB, dtype=np.float32))[None, :], (128, 1))
    return [
        {
            "x": x[b].reshape(128, 512),
            "y": y[b].reshape(128, 512),
            "krow": krow,
        }
        for b in range(B)
    ]


def run(x, y, trace=False, **trace_kw):
    nc = _get_nc()
    res = run_bass_kernel_spmd(nc, _in_maps(x, y), list(range(B)), trace=trace,
                               **trace_kw)
    out = np.stack([res.results[b]["out"] for b in range(B)]).astype(np.float32)
    return out, res


def kernel(x, y):
    out, _ = run(x, y)
    return out


# revision 9
# speedup vs baseline: 1.0969x; 1.0013x over previous
"""Joint soft-histogram kernel v3 for Trainium2 (Bass/Tile), 8-core data parallel.

Math (per batch b, K=256, N=65536 pixels):
    g_j(v) = tanh((640v - 2.5j)/2) = 2*sigmoid(640v - 2.5j) - 1
    M1[k,j] = sum_n g_k(x_n) g_j(y_n),  M2[k,j] = sum_n g_{k+1}(x_n) g_j(y_n)
    out[k,j] = ((M1-M2)[k,j] - (M1-M2)[k,j+1]) / (4N)

Diff-free double accumulation: the x-side adjacent diff (74us of DVE) is
replaced by a second PSUM accumulation whose lhsT is the SAME gx tile sliced
one bin over -- zero extra elementwise work, 4 matmuls/chunk instead of 2.
The busy TensorE warms the PE HAM clock gate to 2.4 GHz, so 2048 matmuls cost
~237us, landing all three engines balanced at ~256/256/237us.

Layout: pixels on partitions, i-major group tiles [128, XG=16, 257] so matmul
operands are contiguous (strided matmul APs measured 2.6-4.6x slower).
Two elementwise paths, split to balance DVE vs ScalarE:
  - grouped: one fp32 DVE tensor_tensor builds arg = 640v - 2.5j via broadcast
    APs (fp16 2x is impossible: the broadcast operand needs innermost stride 0,
    which disqualifies the 2x perf mode), then one big ScalarE ACT
    tanh(0.5*arg) -> fp16 (~3.6us per 4112-elem group).
  - per-chunk (SE only, no DVE): ACT tanh(1.0*krw16h + 320*v[p]) with the
    fp16-exact krow/2 table as input and a per-partition fp32 bias (~500ns per
    257-elem chunk) -- 4 y groups + the 2 tail x groups.
GPSIMD is left idle on purpose: its tensor ops run at ~1.8ns/elem AND slow
concurrent DVE by ~40% (port contention).

Sharding: pure data parallel, batch b -> core b.
"""

import numpy as np

import concourse.bass as bass
import concourse.tile as tile
from concourse import bacc, mybir
from concourse.bass_utils import run_bass_kernel_spmd

F32 = mybir.dt.float32
F16 = mybir.dt.float16

B = 8
K = 256
KB = K + 1
NPIX = 65536
NCHUNK = 512
XG = 16
NG = NCHUNK // XG
SCALE = 640.0
INV = 1.0 / (4.0 * NPIX)

QC0, QC1, QC2 = 0.46564883, -0.02206071, 0.00048341

# --- tuning knobs -----------------------------------------------------------
# per (side, group) sigma path: 'a' = ScalarE ACT tanh, 'q' = custom DVE quintic
SIG_PATH = {}
for g in range(NG):
    SIG_PATH[('x', g)] = 'a'
    SIG_PATH[('y', g)] = 'a'
# per-chunk ACT path (SE-only, no DVE expansion): dict (side, g) -> bool
PCHUNK = {}
for g in range(NG):
    PCHUNK[('y', g)] = g in (2, 7, 12, 17, 22, 27)
    PCHUNK[('x', g)] = g in (31,)
# engine for the x-side diff per group: 'v' (vector) or 'g' (gpsimd)
DIFF_ENG = ['v'] * NG
# ---------------------------------------------------------------------------

_cached_nc = None
_cache_key = None
_tanh_op = None


def _register_tanh_half_op():
    global _tanh_op
    if _tanh_op is not None:
        return _tanh_op
    import concourse.dve_ops as dvo
    from concourse.dve_spec import Spec, Src0, C0, C1, C2, Zero, One, sq, maxx, minn, lower
    from concourse.dve_uop import DveOpSpec

    NAME = "TANH_HALF_QUINT_ANT"
    for op in dvo.OPS:
        if op.name == NAME:
            _tanh_op = op
            return op

    t = sq(Src0)
    poly = Src0 * (C0 + t * (C1 + C2 * t))
    body = minn(maxx(poly, Zero - One), One)

    def _ref(in0, in1, s0, s1, imm2):
        tt = in0 * in0
        return np.clip(in0 * (s0 + tt * (s1 + imm2 * tt)), -1.0, 1.0)

    spec = Spec(body=body, reference=_ref)
    shas = {}
    for ver in ("v3", "v4"):
        uops = lower(spec, ver=ver)
        shas[ver] = DveOpSpec(name=NAME, opcode=1, uops=uops, rd1_en=False).sha(ver)
    op = dvo.DveOp(NAME, spec, subdim=False, uops_sha=shas)
    dvo.OPS.append(op)
    dvo._SUB_OPCODE_FOR_NAME[NAME] = dvo._CUSTOM_DVE_ROW_BASE + len(dvo.OPS) - 1
    assert dvo._SUB_OPCODE_FOR_NAME[NAME] < 0x20
    _tanh_op = op
    return op


def _build():
    op_q = _register_tanh_half_op()
    nc = bacc.Bacc("TRN2")
    xd = nc.declare_dram_parameter("x", [128, 512], F32, isOutput=False)
    yd = nc.declare_dram_parameter("y", [128, 512], F32, isOutput=False)
    kd = nc.declare_dram_parameter("krow", [128, KB], F32, isOutput=False)
    od = nc.declare_dram_parameter("out", [256, 256], F32, isOutput=True)

    tanh = mybir.ActivationFunctionType.Tanh
    sub = mybir.AluOpType.subtract

    with tile.TileContext(nc) as tc:
        with (
            tc.tile_pool(name="singles", bufs=1) as singles,
            tc.tile_pool(name="args", bufs=4) as args,
            tc.tile_pool(name="gs", bufs=6) as gs,
            tc.tile_pool(name="work", bufs=3) as work,
            tc.tile_pool(name="psum", bufs=1, space="PSUM") as psum,
        ):
            warm = singles.tile([128, 8], F16)
            nc.gpsimd.memset(warm, 0.25)
            warm2 = singles.tile([128, 8], F16)
            nc.scalar.activation(out=warm2, in_=warm, func=tanh)

            xt = singles.tile([128, 512], F32)
            nc.sync.dma_start(out=xt, in_=xd[:, :])
            yt = singles.tile([128, 512], F32)
            nc.scalar.dma_start(out=yt, in_=yd[:, :])
            krw = singles.tile([128, KB], F32)
            nc.sync.dma_start(out=krw, in_=kd[:, :])
            krw16h = singles.tile([128, KB], F16)
            nc.scalar.activation(
                out=krw16h, in_=krw,
                func=mybir.ActivationFunctionType.Copy, scale=-0.5,
            )

            x6 = singles.tile([128, 512], F32)
            nc.vector.tensor_scalar_mul(out=x6, in0=xt, scalar1=SCALE)
            x3 = y3 = None
            if any(PCHUNK[('y', g)] for g in range(NG)):
                y3 = singles.tile([128, 512], F32)
                nc.vector.tensor_scalar_mul(out=y3, in0=yt, scalar1=0.5 * SCALE)
            y6 = singles.tile([128, 512], F32)
            nc.vector.tensor_scalar_mul(out=y6, in0=yt, scalar1=SCALE)
            if any(PCHUNK[('x', g)] for g in range(NG)):
                x3 = singles.tile([128, 512], F32)
                nc.vector.tensor_scalar_mul(out=x3, in0=xt, scalar1=0.5 * SCALE)

            M = psum.tile([128, 2, 512], F32)
            M2 = psum.tile([128, 2, 512], F32)

            krb = krw.unsqueeze(1).broadcast_to([128, XG, KB])

            def make_g(side, v6, v3t, g):
                c0 = g * XG
                gt = gs.tile([128, XG, KB], F16, tag=f"g{side}")
                if PCHUNK[(side, g)]:
                    for i in range(XG):
                        nc.scalar.activation(
                            out=gt[:, i, :], in_=krw16h, func=tanh,
                            bias=v3t[:, c0 + i : c0 + i + 1], scale=1.0,
                        )
                    return gt
                arg = args.tile([128, XG, KB], F16, tag=f"arg{side}")
                nc.vector.tensor_tensor(
                    out=arg,
                    in0=v6[:, c0 : c0 + XG].unsqueeze(2).broadcast_to([128, XG, KB]),
                    in1=krb,
                    op=sub,
                )
                if SIG_PATH[(side, g)] == 'a':
                    nc.scalar.activation(out=gt, in_=arg, func=tanh, scale=0.5)
                else:
                    nc.vector._custom_dve(
                        op_q, out=gt[:, :, :], in0=arg[:, :, :],
                        s0=QC0, s1=QC1, imm2=QC2,
                    )
                return gt

            for g in range(NG):
                c0 = g * XG
                if PCHUNK[('y', g)] or PCHUNK[('x', g)]:
                    gy = make_g('y', y6, y3, g)
                    gx = make_g('x', x6, x3, g)
                else:
                    gx = make_g('x', x6, x3, g)
                    gy = make_g('y', y6, y3, g)

                for i in range(XG):
                    c = c0 + i
                    first = c == 0
                    last = c == NCHUNK - 1
                    for h in range(2):
                        nc.tensor.matmul(
                            M[:, h, 0:KB],
                            lhsT=gx[:, i, 128 * h : 128 * h + 128],
                            rhs=gy[:, i, :],
                            start=first,
                            stop=last,
                        )
                        nc.tensor.matmul(
                            M2[:, h, 0:KB],
                            lhsT=gx[:, i, 128 * h + 1 : 128 * h + 129],
                            rhs=gy[:, i, :],
                            start=first,
                            stop=last,
                        )

            for h in range(2):
                m1s = work.tile([128, KB], F32, tag="ep1")
                nc.scalar.activation(
                    out=m1s, in_=M[:, h, 0:KB],
                    func=mybir.ActivationFunctionType.Copy, scale=INV,
                )
                m2s = work.tile([128, KB], F32, tag="ep2")
                nc.scalar.activation(
                    out=m2s, in_=M2[:, h, 0:KB],
                    func=mybir.ActivationFunctionType.Copy, scale=INV,
                )
                t2 = work.tile([128, KB], F32, tag="ep3")
                nc.vector.tensor_sub(out=t2, in0=m1s, in1=m2s)
                t3 = work.tile([128, K], F32, tag="ep4")
                nc.vector.tensor_sub(out=t3, in0=t2[:, 0:K], in1=t2[:, 1:KB])
                nc.sync.dma_start(out=od[128 * h : 128 * (h + 1), :], in_=t3)

    nc.finalize()
    return nc


def _get_nc():
    global _cached_nc, _cache_key
    key = (tuple(sorted(SIG_PATH.items())), tuple(DIFF_ENG), tuple(sorted(PCHUNK.items())))
    if _cached_nc is None or _cache_key != key:
        _cached_nc = _build()
        _cache_key = key
    return _cached_nc


def _in_maps(x, y):
    x = np.ascontiguousarray(np.asarray(x, dtype=np.float32))
    y = np.ascontiguousarray(np.asarray(y, dtype=np.float32))
    krow = np.tile((2.5 * np.arange(KB, dtype=np.float32))[None, :], (128, 1))
    return [
        {
            "x": x[b].reshape(128, 512),
            "y": y[b].reshape(128, 512),
            "krow": krow,
        }
        for b in range(B)
    ]


def run(x, y, trace=False, **trace_kw):
    nc = _get_nc()
    res = run_bass_kernel_spmd(nc, _in_maps(x, y), list(range(B)), trace=trace,
                               **trace_kw)
    out = np.stack([res.results[b]["out"] for b in range(B)]).astype(np.float32)
    return out, res


def kernel(x, y):
    out, _ = run(x, y)
    return out
